# revision 13
# baseline (speedup 1.0000x reference)
"""Trainium2 Bass kernel for nn_GRUEncoderNetwork (GCN + GRU message passing).

Distribution: nodes/edges sharded over 8 NeuronCores by dst-node range.
Per round: each core rebuilds a node-major bf16 table of dis[n]*h[n],
AllGathers it, dma_gathers per-edge source rows (split into two
parity-strided table views to fit int16 gather indices), aggregates per
64-node dst window with one-hot-times-dis matmuls on the PE (feat-major
output), applies the conv weight + ReLU, then runs the GRU cell.
Pooling is a matmul against a precomputed mean-pool matrix followed by an
AllReduce and the output projection.

Self-contained: run by the grading harness as kernel(**inputs).
"""

import math
import os
from contextlib import ExitStack

import numpy as np

import ml_dtypes

import concourse.bacc as bacc
import concourse.bass as bass
import concourse.mybir as mybir
import concourse.tile as tile
from concourse.bass_utils import run_bass_kernel_spmd
from concourse.masks import make_identity

F32 = mybir.dt.float32
F32R = mybir.dt.float32r
BF16 = mybir.dt.bfloat16
I16 = mybir.dt.int16
AF = mybir.ActivationFunctionType
ALU = mybir.AluOpType


class Cfg:
    def __init__(self, N, NE, G, cores, W=64, E=128):
        self.N, self.NE, self.G, self.CORES, self.W, self.E = N, NE, G, cores, W, E
        self.NC = N // cores                      # real nodes per core
        self.NCP = ((self.NC + 127) // 128) * 128  # padded nodes per core
        self.NT = self.NCP // 128                 # 128-node tiles per core
        self.NW = self.NCP // W                   # dst windows per core
        self.NTOT = cores * self.NCP              # global padded rows
        self.HALF = self.NTOT // 2                # rows per parity table
        assert self.HALF <= 32767
        self.R = 4
        self.M2 = 128  # output dim 2*M
        # GRU/conv chunking of the node free dim (>=256 for f32r full rate)
        for c in (448, 512, 384, 256, 128, 64):
            if self.NCP % c == 0:
                self.MCH = c
                break
        self.NMCH = self.NCP // self.MCH


def _divisors(n):
    return [d for d in range(1, n + 1) if n % d == 0]


def build_host(cfg, x, edge_index, batch, W_in, b_in, conv_W, conv_b,
               W_ih, W_hh, b_ih, b_hh, W_out, b_out):
    """All numpy preprocessing. Returns (meta, per-core in_maps list)."""
    N, NC, NCP, W, E, G = cfg.N, cfg.NC, cfg.NCP, cfg.W, cfg.E, cfg.G
    CORES, NW = cfg.CORES, cfg.NW

    src = np.asarray(edge_index[0], dtype=np.int64)
    dst = np.asarray(edge_index[1], dtype=np.int64)
    batch = np.asarray(batch, dtype=np.int64)
    x = np.asarray(x, dtype=np.float32)

    deg = (np.bincount(dst, minlength=N) + 1.0).astype(np.float32)
    dis = (1.0 / np.sqrt(deg)).astype(np.float32)

    # edge list incl. self loops
    a_src = np.concatenate([src, np.arange(N, dtype=np.int64)])
    a_dst = np.concatenate([dst, np.arange(N, dtype=np.int64)])
    gsrc = (a_src // NC) * NCP + (a_src % NC)
    parity = (gsrc & 1).astype(np.int64)
    idx16 = (gsrc >> 1).astype(np.int64)
    sval = dis[a_dst]
    core = a_dst // NC
    lloc = a_dst % NC
    win = lloc // W
    col = lloc % W

    key = (core * NW + win) * 2 + parity
    order = np.argsort(key, kind="stable")
    key_s = key[order]
    # position within run
    runs = CORES * NW * 2
    cnt = np.bincount(key_s, minlength=runs)
    first = np.zeros(runs, dtype=np.int64)
    first[1:] = np.cumsum(cnt)[:-1]
    pos_in_run = np.arange(len(key_s)) - first[key_s]

    cap = int(((cnt.max() + 127) // 128) * 128)
    cpr = cap // 128
    meta = {"cap": cap, "cpr": cpr}

    # call sizing: CALLW windows per gather call, one call per parity
    # HW limit: a single dma_gather call handles at most 1024 indices
    cands = [d for d in _divisors(NW) if d * cap <= 1024]
    callw = max(cands) if cands else 1
    assert callw * cap <= 1024, (callw, cap)
    meta["CALLW"] = callw
    meta["NPAIR"] = NW // callw
    nchunk = NW * 2 * cpr
    meta["NCHUNK"] = nchunk
    t_par = NW * cap

    idx16_s = idx16[order]
    par_s = parity[order]
    win_s = win[order]
    col_s = col[order]
    core_s = core[order]
    sval_s = sval[order]

    # per-core arrays
    in_maps = []
    # shared weights
    w_ihT = np.ascontiguousarray(np.asarray(W_ih, np.float32).T)  # [E, 3E]
    w_hhT = np.ascontiguousarray(np.asarray(W_hh, np.float32).T)
    b_ih = np.asarray(b_ih, np.float32)
    b_hh = np.asarray(b_hh, np.float32)
    gru_bias = np.stack(
        [b_ih[:E] + b_hh[:E], b_ih[E:2 * E] + b_hh[E:2 * E],
         b_hh[2 * E:], b_ih[2 * E:]], axis=1)  # [E, 4] r,z,hn,in

    gcount = np.bincount(batch, minlength=G).astype(np.float32)
    gw = 1.0 / np.maximum(gcount, 1.0)

    for c in range(CORES):
        m = core_s == c
        e_i16 = idx16_s[m]
        e_par = par_s[m]
        e_win = win_s[m]
        e_col = col_s[m]
        e_pos = pos_in_run[m]  # pos_in_run and m are both in sorted-order space

        # gather idx arrays, one per parity, [NW*cap] padded with 0
        idx_flat = np.zeros((2, t_par), dtype=np.int16)
        pos_global = e_win * cap + e_pos
        for p in (0, 1):
            mp = e_par == p
            idx_flat[p, pos_global[mp]] = e_i16[mp].astype(np.int16)
        # wrap: position i -> [i%16, i//16]; replicate to 128 partitions
        cols = t_par // 16
        idx_wrap = np.zeros((2, 128, cols), dtype=np.int16)
        for p in (0, 1):
            wrapped = idx_flat[p].reshape(cols, 16).T  # [16, cols]
            idx_wrap[p] = np.tile(wrapped, (8, 1))

        # S tiles: [128, NCHUNK, W] bf16; chunk = (win*2+par)*cpr + pos//128
        S = np.zeros((128, nchunk, W), dtype=np.float32)
        chunk = (e_win * 2 + e_par) * cpr + e_pos // 128
        row = e_pos % 128
        S[row, chunk, e_col] = sval_s[m]
        S = S.astype(ml_dtypes.bfloat16)

        # x slice transposed [E, NCP]
        xT = np.zeros((E, NCP), dtype=ml_dtypes.bfloat16)
        xT[:, :NC] = x[c * NC:(c + 1) * NC].T.astype(ml_dtypes.bfloat16)

        # dis tiles [128, NT]
        dis_t = np.zeros((128, cfg.NT), dtype=np.float32)
        dl = dis[c * NC:(c + 1) * NC]
        li = np.arange(NC)
        dis_t[li % 128, li // 128] = dl

        # pooling matrix [NT, 128, G]
        P = np.zeros((cfg.NT, 128, G), dtype=np.float32)
        bl = batch[c * NC:(c + 1) * NC]
        P[li // 128, li % 128, bl] = gw[bl]

        in_maps.append({
            "xT": xT,
            "idxA": idx_wrap[0],
            "idxB": idx_wrap[1],
            "S_all": np.asarray(S),
            "P_pool": P,
            "dis_t": dis_t,
            "Win": np.asarray(W_in, np.float32).astype(ml_dtypes.bfloat16),
            "convW": np.concatenate(
                [np.asarray(conv_W, np.float32)[r] for r in range(cfg.R)],
                axis=1).astype(ml_dtypes.bfloat16),
            "convB": np.ascontiguousarray(
                np.asarray(conv_b, np.float32).T),
            "wihT": w_ihT.astype(ml_dtypes.bfloat16),
            "whhT": w_hhT.astype(ml_dtypes.bfloat16),
            "gruB": gru_bias,
            "binp": np.asarray(b_in, np.float32).reshape(E, 1),
            "Wout": np.asarray(W_out, np.float32),
            "bout": np.asarray(b_out, np.float32).reshape(-1, 1),
        })
    return meta, in_maps


def build_bass(cfg, meta, num_devices):
    E, NCP, NT, NW, W, G, R = cfg.E, cfg.NCP, cfg.NT, cfg.NW, cfg.W, cfg.G, cfg.R
    cap, cpr = meta["cap"], meta["cpr"]
    CALLW, NPAIR, NCHUNK = meta["CALLW"], meta["NPAIR"], meta["NCHUNK"]
    t_par = NW * cap
    MCH, NMCH = cfg.MCH, cfg.NMCH
    M2 = cfg.M2

    nc = bacc.Bacc("TRN2", target_bir_lowering=False, debug=False,
                   num_devices=num_devices, num_swdge_queues=4)

    dt_i = {}
    def din(name, shape, dt=F32):
        dt_i[name] = nc.dram_tensor(name, list(shape), dt, kind="ExternalInput")
        return dt_i[name]

    din("xT", [E, NCP], BF16)
    din("idxA", [128, t_par // 16], I16)
    din("idxB", [128, t_par // 16], I16)
    din("S_all", [128, NCHUNK, W], BF16)
    din("P_pool", [NT, 128, G])
    din("dis_t", [128, NT])
    din("Win", [E, E], BF16)
    din("convW", [E, R * E], BF16)
    din("convB", [E, R])
    din("wihT", [E, 3 * E], BF16)
    din("whhT", [E, 3 * E], BF16)
    din("gruB", [E, 4])
    din("binp", [E, 1])
    din("Wout", [E, M2])
    din("bout", [M2, 1])
    outT = nc.dram_tensor("outT", [M2, G], F32, kind="ExternalOutput")

    with tile.TileContext(nc) as tc, ExitStack() as ctx:
        const = ctx.enter_context(tc.tile_pool(name="const", bufs=1))
        state = ctx.enter_context(tc.tile_pool(name="state", bufs=1))
        mpool = ctx.enter_context(tc.tile_pool(name="mpool", bufs=2))
        gtmp = ctx.enter_context(tc.tile_pool(name="gtmp", bufs=2))
        ps_agg = ctx.enter_context(tc.tile_pool(name="ps_agg", bufs=2, space="PSUM"))
        ps_big = ctx.enter_context(tc.tile_pool(name="ps_big", bufs=4, space="PSUM"))
        ps_tr = ctx.enter_context(tc.tile_pool(name="ps_tr", bufs=2, space="PSUM"))
        dram = ctx.enter_context(tc.tile_pool(name="dram", bufs=1, space="DRAM"))

        # ---- constants to SBUF
        def csb(name, shape, dt=F32):
            t = const.tile(list(shape), dt, tag=name)
            nc.sync.dma_start(t[:], dt_i[name][:])
            return t

        win_t = csb("Win", [E, E], BF16)
        convW_t = csb("convW", [E, R * E], BF16)
        convB_t = csb("convB", [E, R])
        wihT_t = csb("wihT", [E, 3 * E], BF16)
        whhT_t = csb("whhT", [E, 3 * E], BF16)
        gruB_t = csb("gruB", [E, 4])
        binp_t = csb("binp", [E, 1])
        wout_t = csb("Wout", [E, M2])
        bout_t = csb("bout", [M2, 1])
        dis_sb = csb("dis_t", [128, NT])
        idxA_t = csb("idxA", [128, t_par // 16], I16)
        idxB_t = csb("idxB", [128, t_par // 16], I16)
        ident = const.tile([128, 128], BF16, tag="ident")
        make_identity(nc, ident[:])

        h_sb = state.tile([E, NCP], BF16, tag="h")

        # DRAM internals
        tbl_in = [dram.tile([NCP, E], BF16, name=f"tbl_in{i}", tag=f"tbl_in{i}") for i in range(R)]
        tbl_full = [dram.tile([cfg.NTOT, E], BF16, name=f"tbl_full{i}", tag=f"tbl_full{i}", addr_space="Shared") for i in range(R)]
        gs_in = dram.tile([E, G], F32, tag="gs_in")
        gs_out = dram.tile([E, G], F32, tag="gs_out", addr_space="Shared")

        rgroup = [list(range(num_devices))]

        # ---- input net: h = relu(Win.T @ xT + b_in)
        xT_sb = state.tile([E, NCP], BF16, tag="agg")  # reused as agg later
        nc.sync.dma_start(xT_sb[:], dt_i["xT"][:])
        for j in range(NMCH):
            sl = slice(j * MCH, (j + 1) * MCH)
            ps = ps_big.tile([128, MCH], F32, space="PSUM", tag="mm")
            nc.tensor.matmul(ps[:], lhsT=win_t[:], rhs=xT_sb[:, sl],
                             start=True, stop=True)
            nc.scalar.activation(h_sb[:, sl], ps[:], AF.Relu, bias=binp_t[:])

        for r in range(R):
            bi = r
            # ---- table build: node-major bf16 dis*h
            slice_sb = mpool.tile([128, NT, E], BF16, tag="slice")
            for t in range(NT):
                trp = ps_tr.tile([128, 128], BF16, space="PSUM", tag="tr")
                nc.tensor.transpose(trp[:], h_sb[:, t * 128:(t + 1) * 128], ident[:])
                nc.scalar.mul(slice_sb[:, t, :], trp[:], dis_sb[:, t:t + 1])
            nc.sync.dma_start(
                tbl_in[bi][:].rearrange("(s p) f -> p s f", p=128), slice_sb[:])
            nc.gpsimd.collective_compute(
                "AllGather", ALU.bypass, replica_groups=rgroup,
                ins=[tbl_in[bi][:].opt()], outs=[tbl_full[bi][:].opt()])

            tblA = tbl_full[bi][:].rearrange("(h two) e -> h (two e)", two=2)[:, 0:E]
            tblB = tbl_full[bi][:].rearrange("(h two) e -> h (two e)", two=2)[:, E:2 * E]

            agg_sb = state.tile([E, NCP], BF16, name=f"agg{r}", tag="agg") if r > 0 else xT_sb
            # ---- gather + aggregate
            for kp in range(NPAIR):
                ccols = CALLW * cap // 16
                nidx = CALLW * cap
                msgs = []
                for p, (idx_t, tbl) in enumerate(((idxA_t, tblA), (idxB_t, tblB))):
                    mt = gtmp.tile([128, CALLW * cpr, E], BF16, tag=f"msgs{p}")
                    nc.gpsimd.dma_gather(
                        out_ap=mt[:], in_ap=tbl,
                        idxs_ap=idx_t[:, kp * ccols:(kp + 1) * ccols],
                        num_idxs=nidx, num_idxs_reg=nidx,
                        elem_size=E, elem_step=2 * E,
                        queue_num=(2 * kp + p) % 4)
                    msgs.append(mt)
                S_t = gtmp.tile([128, CALLW * 2 * cpr, W], BF16, tag="S")
                c0 = kp * CALLW * 2 * cpr
                nc.sync.dma_start(S_t[:], dt_i["S_all"][:, c0:c0 + CALLW * 2 * cpr, :])
                for wl in range(CALLW):
                    w = kp * CALLW + wl
                    ps = ps_agg.tile([128, W], F32, space="PSUM", tag="agg")
                    for p in (0, 1):
                        for j in range(cpr):
                            nc.tensor.matmul(
                                ps[:],
                                lhsT=msgs[p][:, wl * cpr + j, :],
                                rhs=S_t[:, (wl * 2 + p) * cpr + j, :],
                                start=(p == 0 and j == 0),
                                stop=(p == 1 and j == cpr - 1))
                    nc.vector.tensor_copy(out=agg_sb[:, w * W:(w + 1) * W], in_=ps[:])

            # ---- conv: x = relu(convW.T @ agg + b) (in-place over agg_sb)
            for j in range(NMCH):
                sl = slice(j * MCH, (j + 1) * MCH)
                ps = ps_big.tile([128, MCH], F32, space="PSUM", tag="mm")
                nc.tensor.matmul(ps[:], lhsT=convW_t[:, r * E:(r + 1) * E],
                                 rhs=agg_sb[:, sl], start=True, stop=True)
                nc.scalar.activation(agg_sb[:, sl], ps[:], AF.Relu,
                                     bias=convB_t[:, r:r + 1])

            # ---- GRU: h = (1-z)*n + z*h, gates from x(=agg_sb) and h
            for j in range(NMCH):
                sl = slice(j * MCH, (j + 1) * MCH)
                x_ap = agg_sb[:, sl]
                h_ap = h_sb[:, sl]
                ps_r = ps_big.tile([128, MCH], F32, space="PSUM", tag="mm")
                nc.tensor.matmul(ps_r[:], lhsT=wihT_t[:, 0:E],
                                 rhs=x_ap, start=True, stop=False)
                nc.tensor.matmul(ps_r[:], lhsT=whhT_t[:, 0:E],
                                 rhs=h_ap, start=False, stop=True)
                r_t = gtmp.tile([128, MCH], F32, tag="r")
                nc.scalar.activation(r_t[:], ps_r[:], AF.Sigmoid,
                                     bias=gruB_t[:, 0:1])
                ps_z = ps_big.tile([128, MCH], F32, space="PSUM", tag="mm")
                nc.tensor.matmul(ps_z[:], lhsT=wihT_t[:, E:2 * E],
                                 rhs=x_ap, start=True, stop=False)
                nc.tensor.matmul(ps_z[:], lhsT=whhT_t[:, E:2 * E],
                                 rhs=h_ap, start=False, stop=True)
                z_t = gtmp.tile([128, MCH], F32, tag="z")
                nc.scalar.activation(z_t[:], ps_z[:], AF.Sigmoid,
                                     bias=gruB_t[:, 1:2])
                ps_gn = ps_big.tile([128, MCH], F32, space="PSUM", tag="mm")
                nc.tensor.matmul(ps_gn[:], lhsT=wihT_t[:, 2 * E:3 * E],
                                 rhs=x_ap, start=True, stop=True)
                ps_hn = ps_big.tile([128, MCH], F32, space="PSUM", tag="mm")
                nc.tensor.matmul(ps_hn[:], lhsT=whhT_t[:, 2 * E:3 * E],
                                 rhs=h_ap, start=True, stop=True)
                u_t = gtmp.tile([128, MCH], F32, tag="u")
                nc.vector.tensor_scalar(out=u_t[:], in0=ps_hn[:],
                                        scalar1=gruB_t[:, 2:3], scalar2=None,
                                        op0=ALU.add)
                nc.vector.tensor_tensor(out=u_t[:], in0=r_t[:], in1=u_t[:],
                                        op=ALU.mult)
                nc.vector.tensor_tensor(out=u_t[:], in0=ps_gn[:], in1=u_t[:],
                                        op=ALU.add)
                n_t = gtmp.tile([128, MCH], F32, tag="n")
                nc.scalar.activation(n_t[:], u_t[:], AF.Tanh,
                                     bias=gruB_t[:, 3:4])
                s_t = gtmp.tile([128, MCH], F32, tag="s")
                nc.vector.tensor_tensor(out=s_t[:], in0=h_sb[:, sl], in1=n_t[:],
                                        op=ALU.subtract)
                nc.vector.tensor_tensor(out=s_t[:], in0=z_t[:], in1=s_t[:],
                                        op=ALU.mult)
                nc.vector.tensor_tensor(out=h_sb[:, sl], in0=n_t[:], in1=s_t[:],
                                        op=ALU.add)

        # ---- pooling: gs = sum_t hnode_t.T @ P_t  -> [E, G] feat-major
        gs_ps = ps_big.tile([128, G], F32, space="PSUM", tag="mm")
        zro = gtmp.tile([128, G], BF16, tag="zro")
        nc.vector.memset(zro[:], 0.0)
        nc.tensor.matmul(gs_ps[:], lhsT=ident[:], rhs=zro[:], start=True, stop=False)
        for t in range(NT):
            trp = ps_tr.tile([128, 128], BF16, space="PSUM", tag="tr")
            nc.tensor.transpose(trp[:], h_sb[:, t * 128:(t + 1) * 128], ident[:])
            hn_t = gtmp.tile([128, 128], F32, tag="hn")
            nc.vector.tensor_copy(out=hn_t[:], in_=trp[:])
            p_t = gtmp.tile([128, G], F32, tag="pp")
            nc.sync.dma_start(p_t[:], dt_i["P_pool"][t, :, :])
            nc.tensor.matmul(gs_ps[:], lhsT=hn_t[:], rhs=p_t[:],
                             start=False, stop=(t == NT - 1))
        gs_sb = gtmp.tile([E, G], F32, tag="gs")
        nc.vector.tensor_copy(out=gs_sb[:], in_=gs_ps[:])
        nc.sync.dma_start(gs_in[:], gs_sb[:])
        nc.gpsimd.collective_compute(
            "AllReduce", ALU.add, replica_groups=rgroup,
            ins=[gs_in[:].opt()], outs=[gs_out[:].opt()])
        gs_red = gtmp.tile([E, G], F32, tag="gsr")
        nc.sync.dma_start(gs_red[:], gs_out[:])
        ps_o = ps_big.tile([128, G], F32, space="PSUM", tag="mm")
        nc.tensor.matmul(ps_o[:], lhsT=wout_t[:], rhs=gs_red[:],
                         start=True, stop=True)
        out_sb = gtmp.tile([M2, G], F32, tag="out")
        nc.scalar.activation(out_sb[:], ps_o[:], AF.Identity, bias=bout_t[:])
        nc.sync.dma_start(outT[:], out_sb[:])

    nc.compile()
    return nc


# ---------------- full-size entry point ----------------

_CACHE = {}


def _get_full():
    if "full" not in _CACHE:
        _CACHE["full"] = Cfg(N=50000, NE=600000, G=512, cores=8, W=64)
    return _CACHE["full"]


def kernel(x, edge_index, batch, W_in, b_in, conv_W, conv_b,
           W_ih, W_hh, b_ih, b_hh, W_out, b_out):
    cfg = _get_full()
    meta, in_maps = build_host(cfg, x, edge_index, batch, W_in, b_in,
                               conv_W, conv_b, W_ih, W_hh, b_ih, b_hh,
                               W_out, b_out)
    key = ("bass", cfg.N, meta["cap"])
    if key not in _CACHE:
        _CACHE[key] = build_bass(cfg, meta, cfg.CORES)
    nc = _CACHE[key]
    trace = bool(int(os.environ.get("GNN_TRACE", "0")))
    res = run_bass_kernel_spmd(nc, in_maps, core_ids=list(range(cfg.CORES)),
                               trace=trace)
    _CACHE["last_results"] = res
    out = np.asarray(res.results[0]["outT"])  # [2M, G]
    return np.ascontiguousarray(out.T[:cfg.G]).astype(np.float32)


# revision 23
# speedup vs baseline: 51.7030x; 51.7030x over previous
"""Trainium2 Bass kernel for nn_GRUEncoderNetwork (GCN + GRU message passing).

Distribution: nodes/edges sharded over 8 NeuronCores by dst-node range.
Per round: each core rebuilds a node-major bf16 table of dis[n]*h[n],
AllGathers it, dma_gathers per-edge source rows (split into two
parity-strided table views to fit int16 gather indices), aggregates per
64-node dst window with one-hot-times-dis matmuls on the PE (feat-major
output), applies the conv weight + ReLU, then runs the GRU cell.
Pooling is a matmul against a precomputed mean-pool matrix followed by an
AllReduce and the output projection.

Self-contained: run by the grading harness as kernel(**inputs).
"""

import math
import os
from contextlib import ExitStack

import numpy as np

import ml_dtypes

import concourse.bacc as bacc
import concourse.bass as bass
import concourse.mybir as mybir
import concourse.tile as tile
from concourse.bass_utils import run_bass_kernel_spmd
from concourse.masks import make_identity

F32 = mybir.dt.float32
F32R = mybir.dt.float32r
BF16 = mybir.dt.bfloat16
I16 = mybir.dt.int16
AF = mybir.ActivationFunctionType
ALU = mybir.AluOpType


class Cfg:
    def __init__(self, N, NE, G, cores, W=64, E=128):
        self.N, self.NE, self.G, self.CORES, self.W, self.E = N, NE, G, cores, W, E
        self.NC = N // cores                      # real nodes per core
        self.NCP = ((self.NC + 127) // 128) * 128  # padded nodes per core
        self.NT = self.NCP // 128                 # 128-node tiles per core
        self.NW = self.NCP // W                   # dst windows per core
        self.NTOT = cores * self.NCP              # global padded rows
        self.HALF = self.NTOT // 2                # rows per parity table
        assert self.HALF <= 32767
        self.R = 4
        self.M2 = 128  # output dim 2*M
        # GRU/conv chunking of the node free dim (>=256 for f32r full rate)
        for c in (448, 512, 384, 256, 128, 64):
            if self.NCP % c == 0:
                self.MCH = c
                break
        self.NMCH = self.NCP // self.MCH


def _divisors(n):
    return [d for d in range(1, n + 1) if n % d == 0]


def build_host(cfg, x, edge_index, batch, W_in, b_in, conv_W, conv_b,
               W_ih, W_hh, b_ih, b_hh, W_out, b_out):
    """All numpy preprocessing. Returns (meta, per-core in_maps list)."""
    N, NC, NCP, W, E, G = cfg.N, cfg.NC, cfg.NCP, cfg.W, cfg.E, cfg.G
    CORES, NW = cfg.CORES, cfg.NW

    src = np.asarray(edge_index[0], dtype=np.int64)
    dst = np.asarray(edge_index[1], dtype=np.int64)
    batch = np.asarray(batch, dtype=np.int64)
    x = np.asarray(x, dtype=np.float32)

    deg = (np.bincount(dst, minlength=N) + 1.0).astype(np.float32)
    dis = (1.0 / np.sqrt(deg)).astype(np.float32)

    # edge list incl. self loops
    a_src = np.concatenate([src, np.arange(N, dtype=np.int64)])
    a_dst = np.concatenate([dst, np.arange(N, dtype=np.int64)])
    gsrc = (a_src // NC) * NCP + (a_src % NC)
    parity = (gsrc & 1).astype(np.int64)
    idx16 = (gsrc >> 1).astype(np.int64)
    sval = dis[a_dst]
    core = a_dst // NC
    lloc = a_dst % NC
    win = lloc // W
    col = lloc % W

    key = (core * NW + win) * 2 + parity
    order = np.argsort(key, kind="stable")
    key_s = key[order]
    # position within run
    runs = CORES * NW * 2
    cnt = np.bincount(key_s, minlength=runs)
    first = np.zeros(runs, dtype=np.int64)
    first[1:] = np.cumsum(cnt)[:-1]
    pos_in_run = np.arange(len(key_s)) - first[key_s]

    cap = int(((cnt.max() + 127) // 128) * 128)
    cpr = cap // 128
    meta = {"cap": cap, "cpr": cpr}

    # call sizing: CALLW windows per gather call, one call per parity
    # HW limit: a single dma_gather call handles at most 1024 indices
    cands = [d for d in _divisors(NW) if d * cap <= 1024]
    callw = max(cands) if cands else 1
    assert callw * cap <= 1024, (callw, cap)
    meta["CALLW"] = callw
    meta["NPAIR"] = NW // callw
    nchunk = NW * 2 * cpr
    meta["NCHUNK"] = nchunk
    t_par = NW * cap

    idx16_s = idx16[order]
    par_s = parity[order]
    win_s = win[order]
    col_s = col[order]
    core_s = core[order]
    sval_s = sval[order]

    # per-core arrays
    in_maps = []
    # shared weights
    w_ihT = np.ascontiguousarray(np.asarray(W_ih, np.float32).T)  # [E, 3E]
    w_hhT = np.ascontiguousarray(np.asarray(W_hh, np.float32).T)
    b_ih = np.asarray(b_ih, np.float32)
    b_hh = np.asarray(b_hh, np.float32)
    gru_bias = np.stack(
        [b_ih[:E] + b_hh[:E], b_ih[E:2 * E] + b_hh[E:2 * E],
         b_hh[2 * E:], b_ih[2 * E:]], axis=1)  # [E, 4] r,z,hn,in

    gcount = np.bincount(batch, minlength=G).astype(np.float32)
    gw = 1.0 / np.maximum(gcount, 1.0)

    for c in range(CORES):
        m = core_s == c
        e_i16 = idx16_s[m]
        e_par = par_s[m]
        e_win = win_s[m]
        e_col = col_s[m]
        e_pos = pos_in_run[m]  # pos_in_run and m are both in sorted-order space

        # gather idx arrays, one per parity, [NW*cap] padded with 0
        idx_flat = np.zeros((2, t_par), dtype=np.int16)
        pos_global = e_win * cap + e_pos
        for p in (0, 1):
            mp = e_par == p
            idx_flat[p, pos_global[mp]] = e_i16[mp].astype(np.int16)
        # wrap: position i -> [i%16, i//16]; replicate to 128 partitions
        cols = t_par // 16
        idx_wrap = np.zeros((2, 128, cols), dtype=np.int16)
        for p in (0, 1):
            wrapped = idx_flat[p].reshape(cols, 16).T  # [16, cols]
            idx_wrap[p] = np.tile(wrapped, (8, 1))

        # S tiles: [128, NCHUNK, W] bf16; chunk = (win*2+par)*cpr + pos//128
        S = np.zeros((128, nchunk, W), dtype=np.float32)
        chunk = (e_win * 2 + e_par) * cpr + e_pos // 128
        row = e_pos % 128
        S[row, chunk, e_col] = sval_s[m]
        S = S.astype(ml_dtypes.bfloat16)

        # x slice transposed [E, NCP]
        xT = np.zeros((E, NCP), dtype=ml_dtypes.bfloat16)
        xT[:, :NC] = x[c * NC:(c + 1) * NC].T.astype(ml_dtypes.bfloat16)

        # dis tiles [128, NT]
        dis_t = np.zeros((128, cfg.NT), dtype=np.float32)
        dl = dis[c * NC:(c + 1) * NC]
        li = np.arange(NC)
        dis_t[li % 128, li // 128] = dl

        # pooling matrix [NT, 128, G]
        P = np.zeros((cfg.NT, 128, G), dtype=np.float32)
        bl = batch[c * NC:(c + 1) * NC]
        P[li // 128, li % 128, bl] = gw[bl]

        in_maps.append({
            "xT": xT,
            "idxA": idx_wrap[0],
            "idxB": idx_wrap[1],
            "S_all": np.asarray(S),
            "P_pool": P,
            "dis_t": dis_t,
            "Win": np.asarray(W_in, np.float32).astype(ml_dtypes.bfloat16),
            "convW": np.concatenate(
                [np.asarray(conv_W, np.float32)[r] for r in range(cfg.R)],
                axis=1).astype(ml_dtypes.bfloat16),
            "convB": np.ascontiguousarray(
                np.asarray(conv_b, np.float32).T),
            "wihT": w_ihT.astype(ml_dtypes.bfloat16),
            "whhT": w_hhT.astype(ml_dtypes.bfloat16),
            "gruB": gru_bias,
            "binp": np.asarray(b_in, np.float32).reshape(E, 1),
            "Wout": np.asarray(W_out, np.float32),
            "bout": np.asarray(b_out, np.float32).reshape(-1, 1),
        })
    return meta, in_maps


def build_bass(cfg, meta, num_devices, rounds_mult=1, phases=None):
    phases = phases or {"tbl", "ag", "agg", "gath", "conv", "gru"}
    E, NCP, NT, NW, W, G, R = cfg.E, cfg.NCP, cfg.NT, cfg.NW, cfg.W, cfg.G, cfg.R
    cap, cpr = meta["cap"], meta["cpr"]
    CALLW, NPAIR, NCHUNK = meta["CALLW"], meta["NPAIR"], meta["NCHUNK"]
    t_par = NW * cap
    MCH, NMCH = cfg.MCH, cfg.NMCH
    M2 = cfg.M2

    nc = bacc.Bacc("TRN2", target_bir_lowering=False, debug=False,
                   num_devices=num_devices, num_swdge_queues=4)

    dt_i = {}
    def din(name, shape, dt=F32):
        dt_i[name] = nc.dram_tensor(name, list(shape), dt, kind="ExternalInput")
        return dt_i[name]

    din("xT", [E, NCP], BF16)
    din("idxA", [128, t_par // 16], I16)
    din("idxB", [128, t_par // 16], I16)
    din("S_all", [128, NCHUNK, W], BF16)
    din("P_pool", [NT, 128, G])
    din("dis_t", [128, NT])
    din("Win", [E, E], BF16)
    din("convW", [E, R * E], BF16)
    din("convB", [E, R])
    din("wihT", [E, 3 * E], BF16)
    din("whhT", [E, 3 * E], BF16)
    din("gruB", [E, 4])
    din("binp", [E, 1])
    din("Wout", [E, M2])
    din("bout", [M2, 1])
    outT = nc.dram_tensor("outT", [M2, G], F32, kind="ExternalOutput")

    with tile.TileContext(nc) as tc, ExitStack() as ctx:
        const = ctx.enter_context(tc.tile_pool(name="const", bufs=1))
        state = ctx.enter_context(tc.tile_pool(name="state", bufs=1))
        mpool = ctx.enter_context(tc.tile_pool(name="mpool", bufs=2))
        gtmp = ctx.enter_context(tc.tile_pool(name="gtmp", bufs=4))
        ps_agg = ctx.enter_context(tc.tile_pool(name="ps_agg", bufs=3, space="PSUM"))
        ps_big = ctx.enter_context(tc.tile_pool(name="ps_big", bufs=4, space="PSUM"))
        ps_tr = ctx.enter_context(tc.tile_pool(name="ps_tr", bufs=1, space="PSUM"))
        dram = ctx.enter_context(tc.tile_pool(name="dram", bufs=1, space="DRAM"))

        # ---- constants to SBUF
        def csb(name, shape, dt=F32):
            t = const.tile(list(shape), dt, tag=name)
            nc.sync.dma_start(t[:], dt_i[name][:])
            return t

        win_t = csb("Win", [E, E], BF16)
        convW_t = csb("convW", [E, R * E], BF16)
        convB_t = csb("convB", [E, R])
        wihT_t = csb("wihT", [E, 3 * E], BF16)
        whhT_t = csb("whhT", [E, 3 * E], BF16)
        gruB_t = csb("gruB", [E, 4])
        binp_t = csb("binp", [E, 1])
        wout_t = csb("Wout", [E, M2])
        bout_t = csb("bout", [M2, 1])
        dis_sb = csb("dis_t", [128, NT])
        idxA_t = csb("idxA", [128, t_par // 16], I16)
        idxB_t = csb("idxB", [128, t_par // 16], I16)
        ident = const.tile([128, 128], BF16, tag="ident")
        make_identity(nc, ident[:])

        h_sb = state.tile([E, NCP], BF16, tag="h")

        # DRAM internals
        NRR = R * rounds_mult
        tbl_in = [dram.tile([NCP, E], BF16, name=f"tbl_in{i}", tag=f"tbl_in{i}") for i in range(NRR)]
        tbl_full = [dram.tile([cfg.NTOT, E], BF16, name=f"tbl_full{i}", tag=f"tbl_full{i}", addr_space="Shared") for i in range(NRR)]
        gs_in = dram.tile([E, G], F32, tag="gs_in")
        gs_out = dram.tile([E, G], F32, tag="gs_out", addr_space="Shared")

        rgroup = [list(range(num_devices))]

        # ---- input net: h = relu(Win.T @ xT + b_in)
        xT_sb = state.tile([E, NCP], BF16, tag="agg")  # reused as agg later
        nc.sync.dma_start(xT_sb[:], dt_i["xT"][:])
        for j in range(NMCH):
            sl = slice(j * MCH, (j + 1) * MCH)
            ps = ps_big.tile([128, MCH], F32, space="PSUM", tag="mm")
            nc.tensor.matmul(ps[:], lhsT=win_t[:], rhs=xT_sb[:, sl],
                             start=True, stop=True)
            nc.scalar.activation(h_sb[:, sl], ps[:], AF.Relu, bias=binp_t[:])

        for rr in range(R * rounds_mult):
            r = rr % R
            bi = rr
            # ---- table build: node-major bf16 dis*h
            if "tbl" in phases:
                slice_sb = mpool.tile([128, NT, E], BF16, tag="slice")
                for t in range(NT):
                    trp = ps_tr.tile([128, 128], BF16, space="PSUM", tag="tr")
                    nc.tensor.transpose(trp[:], h_sb[:, t * 128:(t + 1) * 128], ident[:])
                    nc.scalar.mul(slice_sb[:, t, :], trp[:], dis_sb[:, t:t + 1])
                nc.sync.dma_start(
                    tbl_in[bi][:].rearrange("(s p) f -> p s f", p=128), slice_sb[:])
                if "ag" in phases:
                    if num_devices > 1:
                        nc.gpsimd.collective_compute(
                            "AllGather", ALU.bypass, replica_groups=rgroup,
                            ins=[tbl_in[bi][:].opt()], outs=[tbl_full[bi][:].opt()])
                    else:
                        nc.gpsimd.dma_start(tbl_full[bi][:NCP, :], tbl_in[bi][:])

            tblA = tbl_full[bi][:].rearrange("(h two) e -> h (two e)", two=2)[:, 0:E]
            tblB = tbl_full[bi][:].rearrange("(h two) e -> h (two e)", two=2)[:, E:2 * E]

            agg_sb = state.tile([E, NCP], BF16, name=f"agg{rr}", tag="agg") if rr > 0 else xT_sb
            # ---- gather + aggregate
            for kp in range(NPAIR):
                ccols = CALLW * cap // 16
                nidx = CALLW * cap
                msgs = []
                for p, (idx_t, tbl) in enumerate(((idxA_t, tblA), (idxB_t, tblB))):
                    mt = gtmp.tile([128, CALLW * cpr, E], BF16, tag=f"msgs{p}")
                    if "gath" not in phases:
                        nc.vector.memset(mt[:, 0, :], 0)
                    if "gath" in phases:
                        nc.gpsimd.dma_gather(
                            out_ap=mt[:], in_ap=tbl,
                            idxs_ap=idx_t[:, kp * ccols:(kp + 1) * ccols],
                            num_idxs=nidx, num_idxs_reg=nidx,
                            elem_size=E, elem_step=2 * E,
                            queue_num=(2 * kp + p) % 4)
                    msgs.append(mt)
                S_t = gtmp.tile([128, CALLW * 2 * cpr, W], BF16, tag="S")
                c0 = kp * CALLW * 2 * cpr
                if "agg" not in phases:
                    continue
                nc.sync.dma_start(S_t[:], dt_i["S_all"][:, c0:c0 + CALLW * 2 * cpr, :])
                for wl in range(CALLW):
                    w = kp * CALLW + wl
                    ps = ps_agg.tile([128, W], F32, space="PSUM", tag="agg")
                    for p in (0, 1):
                        for j in range(cpr):
                            nc.tensor.matmul(
                                ps[:],
                                lhsT=msgs[p][:, wl * cpr + j, :],
                                rhs=S_t[:, (wl * 2 + p) * cpr + j, :],
                                start=(p == 0 and j == 0),
                                stop=(p == 1 and j == cpr - 1))
                    nc.vector.tensor_copy(out=agg_sb[:, w * W:(w + 1) * W], in_=ps[:])

            # ---- conv: x = relu(convW.T @ agg + b) (in-place over agg_sb)
            for j in range(NMCH if "conv" in phases else 0):
                sl = slice(j * MCH, (j + 1) * MCH)
                ps = ps_big.tile([128, MCH], F32, space="PSUM", tag="mm")
                nc.tensor.matmul(ps[:], lhsT=convW_t[:, r * E:(r + 1) * E],
                                 rhs=agg_sb[:, sl], start=True, stop=True)
                nc.scalar.activation(agg_sb[:, sl], ps[:], AF.Relu,
                                     bias=convB_t[:, r:r + 1])

            # ---- GRU: h = (1-z)*n + z*h, gates from x(=agg_sb) and h
            if "gru" not in phases:
                for j in range(NMCH):
                    sl = slice(j * MCH, (j + 1) * MCH)
                    nc.vector.tensor_copy(out=h_sb[:, sl], in_=agg_sb[:, sl])
                continue
            for j in range(NMCH):
                sl = slice(j * MCH, (j + 1) * MCH)
                x_ap = agg_sb[:, sl]
                h_ap = h_sb[:, sl]
                ps_r = ps_big.tile([128, MCH], F32, space="PSUM", tag="mm")
                nc.tensor.matmul(ps_r[:], lhsT=wihT_t[:, 0:E],
                                 rhs=x_ap, start=True, stop=False)
                nc.tensor.matmul(ps_r[:], lhsT=whhT_t[:, 0:E],
                                 rhs=h_ap, start=False, stop=True)
                r_t = gtmp.tile([128, MCH], F32, tag="r")
                nc.scalar.activation(r_t[:], ps_r[:], AF.Sigmoid,
                                     bias=gruB_t[:, 0:1])
                ps_z = ps_big.tile([128, MCH], F32, space="PSUM", tag="mm")
                nc.tensor.matmul(ps_z[:], lhsT=wihT_t[:, E:2 * E],
                                 rhs=x_ap, start=True, stop=False)
                nc.tensor.matmul(ps_z[:], lhsT=whhT_t[:, E:2 * E],
                                 rhs=h_ap, start=False, stop=True)
                z_t = gtmp.tile([128, MCH], F32, tag="z")
                nc.scalar.activation(z_t[:], ps_z[:], AF.Sigmoid,
                                     bias=gruB_t[:, 1:2])
                ps_gn = ps_big.tile([128, MCH], F32, space="PSUM", tag="mm")
                nc.tensor.matmul(ps_gn[:], lhsT=wihT_t[:, 2 * E:3 * E],
                                 rhs=x_ap, start=True, stop=True)
                ps_hn = ps_big.tile([128, MCH], F32, space="PSUM", tag="mm")
                nc.tensor.matmul(ps_hn[:], lhsT=whhT_t[:, 2 * E:3 * E],
                                 rhs=h_ap, start=True, stop=True)
                u_t = gtmp.tile([128, MCH], F32, tag="u")
                nc.vector.tensor_scalar(out=u_t[:], in0=ps_hn[:],
                                        scalar1=gruB_t[:, 2:3], scalar2=None,
                                        op0=ALU.add)
                nc.vector.tensor_tensor(out=u_t[:], in0=r_t[:], in1=u_t[:],
                                        op=ALU.mult)
                nc.vector.tensor_tensor(out=u_t[:], in0=ps_gn[:], in1=u_t[:],
                                        op=ALU.add)
                n_t = gtmp.tile([128, MCH], F32, tag="n")
                nc.scalar.activation(n_t[:], u_t[:], AF.Tanh,
                                     bias=gruB_t[:, 3:4])
                s_t = gtmp.tile([128, MCH], F32, tag="s")
                nc.vector.tensor_tensor(out=s_t[:], in0=h_sb[:, sl], in1=n_t[:],
                                        op=ALU.subtract)
                nc.vector.tensor_tensor(out=s_t[:], in0=z_t[:], in1=s_t[:],
                                        op=ALU.mult)
                nc.vector.tensor_tensor(out=h_sb[:, sl], in0=n_t[:], in1=s_t[:],
                                        op=ALU.add)

        # ---- pooling: gs = sum_t hnode_t.T @ P_t  -> [E, G] feat-major
        gs_ps = ps_big.tile([128, G], F32, space="PSUM", tag="mm")
        zro = gtmp.tile([128, G], BF16, tag="zro")
        nc.vector.memset(zro[:], 0.0)
        nc.tensor.matmul(gs_ps[:], lhsT=ident[:], rhs=zro[:], start=True, stop=False)
        for t in range(NT):
            trp = ps_tr.tile([128, 128], BF16, space="PSUM", tag="tr")
            nc.tensor.transpose(trp[:], h_sb[:, t * 128:(t + 1) * 128], ident[:])
            hn_t = gtmp.tile([128, 128], F32, tag="hn")
            nc.vector.tensor_copy(out=hn_t[:], in_=trp[:])
            p_t = gtmp.tile([128, G], F32, tag="pp")
            nc.sync.dma_start(p_t[:], dt_i["P_pool"][t, :, :])
            nc.tensor.matmul(gs_ps[:], lhsT=hn_t[:], rhs=p_t[:],
                             start=False, stop=(t == NT - 1))
        gs_sb = gtmp.tile([E, G], F32, tag="gs")
        nc.vector.tensor_copy(out=gs_sb[:], in_=gs_ps[:])
        nc.sync.dma_start(gs_in[:], gs_sb[:])
        if num_devices > 1:
            nc.gpsimd.collective_compute(
                "AllReduce", ALU.add, replica_groups=rgroup,
                ins=[gs_in[:].opt()], outs=[gs_out[:].opt()])
        else:
            nc.gpsimd.dma_start(gs_out[:], gs_in[:])
        gs_red = gtmp.tile([E, G], F32, tag="gsr")
        nc.sync.dma_start(gs_red[:], gs_out[:])
        ps_o = ps_big.tile([128, G], F32, space="PSUM", tag="mm")
        nc.tensor.matmul(ps_o[:], lhsT=wout_t[:], rhs=gs_red[:],
                         start=True, stop=True)
        out_sb = gtmp.tile([M2, G], F32, tag="out")
        nc.scalar.activation(out_sb[:], ps_o[:], AF.Identity, bias=bout_t[:])
        nc.sync.dma_start(outT[:], out_sb[:])

    nc.compile()
    return nc


# ---------------- full-size entry point ----------------

_CACHE = {}


def _get_full():
    if "full" not in _CACHE:
        _CACHE["full"] = Cfg(N=50000, NE=600000, G=512, cores=8, W=64)
    return _CACHE["full"]


def kernel(x, edge_index, batch, W_in, b_in, conv_W, conv_b,
           W_ih, W_hh, b_ih, b_hh, W_out, b_out):
    cfg = _get_full()
    meta, in_maps = build_host(cfg, x, edge_index, batch, W_in, b_in,
                               conv_W, conv_b, W_ih, W_hh, b_ih, b_hh,
                               W_out, b_out)
    key = ("bass", cfg.N, meta["cap"])
    if key not in _CACHE:
        _CACHE[key] = build_bass(cfg, meta, cfg.CORES)
    nc = _CACHE[key]
    trace = bool(int(os.environ.get("GNN_TRACE", "0")))
    res = run_bass_kernel_spmd(nc, in_maps, core_ids=list(range(cfg.CORES)),
                               trace=trace)
    _CACHE["last_results"] = res
    out = np.asarray(res.results[0]["outT"])  # [2M, G]
    return np.ascontiguousarray(out.T[:cfg.G]).astype(np.float32)
